# revision 17
# baseline (speedup 1.0000x reference)
"""Trainium2 Bass kernel for AverageSpanExtractor (segment mean over spans).

Math note: the reference's masked softmax over all-ones logits reduces
exactly to a mean over the span tokens [start, end):
    out[b, n, :] = mean(sequence_tensor[b, start:end, :]).

Strategy (8 cores, batch-parallel — one batch element per core):
  1. Block-local inclusive prefix sums via fp32r triangular matmul
     (1 cycle/row at free dim 256), cast fp16 into an SBUF-resident
     rank-stripe table `cum`: gather-token t -> partition t&127, rank
     t>>7 (512 B per rank row). Rank 0 is zeros; rank k+1 holds block
     k's prefixes. No DRAM round-trip.
  2. One SWDGE dma_gather (SBUF source, transpose mode) fetches, for
     every span n, three fp16 rows:
       CE = cum[e+127], CS = cum[s+127], CR = cum[(gb<<7)+127],
     gb = (qe-qs)*qe with qe=(e-1)>>7, qs=(s-1)>>7.  CR is the
     crossed-block total (rank-0 zeros when the span stays in-block).
     span_sum = CE - CS + CR  (the +127 shift makes row s=0 hit the
     zero rank; block-start rows self-correct via CR).
     Output arrives transposed: [d%128, d>>7, span] — combine runs in
     fp16 at 2x DVE rate.
  3. Per 128-span chunk: PE transpose back to [span, d], scale by 1/w
     (per-partition scalar) during the PSUM->SBUF copy, store fp32.

Precision: fp16 table (2^-11 rel quant on block-local values of std
<= 11) + fp32r matmul => ~1.5e-3 global rel err, well under the 2e-2
gate.
"""

import numpy as np

B, S, D = 8, 4096, 256
N_SPANS = 1024
P = 128
NBLK = S // P          # 32 blocks of 128 tokens
NRANK = NBLK + 1       # +1 zero rank at the front
NI = 3 * N_SPANS       # gather indices: e, s, correction
JG = N_SPANS // P      # 8 span chunks of 128

_TRI = np.triu(np.ones((P, P), dtype=np.float32))

_cached_nc = None


def build_nc():
    import concourse.bass as bass
    import concourse.bacc as bacc
    import concourse.mybir as mybir
    from concourse.tile import TileContext
    from concourse.masks import make_identity
    from concourse import library_config

    f32 = mybir.dt.float32
    f32r = mybir.dt.float32r
    f16 = mybir.dt.float16
    i16 = mybir.dt.int16
    i32 = mybir.dt.int32
    Alu = mybir.AluOpType
    Act = mybir.ActivationFunctionType

    nc = bacc.Bacc(
        None, target_bir_lowering=False, debug=False, num_devices=B,
        num_swdge_queues=4,
    )
    seq = nc.declare_dram_parameter("seq", [S, D], f32, isOutput=False)
    trid = nc.declare_dram_parameter("trid", [P, P], f32, isOutput=False)
    # Wrapped SWDGE index layout, replicated across the 8 gpsimd cores:
    # logical idx i lives at [i%16 (mod 16), i//16]. Per span half h
    # (512 spans): 32 slots raw e, then 32 slots raw s. int32 because
    # DVE shifts are 32-bit-only; cast to the int16 idx tile after the
    # index math.
    sew = nc.declare_dram_parameter("sew", [P, 128], i32, isOutput=False)
    # Span (s, e) pairs in output layout: span n=c*128+p at [p, 2c], [p, 2c+1].
    vse = nc.declare_dram_parameter("vse", [P, 2 * JG], i32, isOutput=False)
    out = nc.declare_dram_parameter("out", [N_SPANS, D], f32, isOutput=True)

    with TileContext(nc) as tc:
        with (
            tc.tile_pool(name="const", bufs=1) as const_pool,
            tc.tile_pool(name="x", bufs=4) as x_pool,
            tc.tile_pool(name="ps", bufs=4, space="PSUM") as ps_pool,
            tc.tile_pool(name="pst", bufs=4, space="PSUM") as pst_pool,
            tc.tile_pool(name="misc", bufs=1) as misc_pool,
            tc.tile_pool(name="res", bufs=3) as res_pool,
        ):
            # --- constants ---
            # tri/bigx live as f32r: walrus's verifier requires every producer
            # of an fp32r matmul input to emit fp32r-typed output, and memset/
            # affine_select can't write f32r — so tri ships from the host.
            tri = const_pool.tile([P, P], f32r)
            nc.sync.dma_start(out=tri[:], in_=trid[:].bitcast(f32r))
            identh = const_pool.tile([P, P], f16)
            make_identity(nc, identh[:])
            nc.gpsimd.load_library(library_config.mlp)

            # --- span index prep (overlaps the seq loads) ---
            # idx slot map per half h: [96h, 96h+32) = e+127,
            # [96h+32, 96h+64) = s+127, [96h+64, 96h+96) = gb*128+127.
            sew32 = misc_pool.tile([P, 128], i32)
            nc.sync.dma_start(out=sew32[:], in_=sew[:])
            idx = misc_pool.tile([P, NI // 16], i16)
            for h in range(2):
                E32 = sew32[:, 64 * h : 64 * h + 32]
                S32 = sew32[:, 64 * h + 32 : 64 * h + 64]
                nc.vector.tensor_scalar(
                    out=E32, in0=E32, scalar1=127, scalar2=None, op0=Alu.add
                )
                nc.vector.tensor_scalar(
                    out=S32, in0=S32, scalar1=127, scalar2=None, op0=Alu.add
                )
                nc.vector.tensor_copy(
                    out=idx[:, 96 * h : 96 * h + 64],
                    in_=sew32[:, 64 * h : 64 * h + 64],
                )
                qe1 = misc_pool.tile([P, 32], i32, name=f"qe1{h}")
                nc.vector.tensor_scalar(
                    out=qe1[:], in0=E32, scalar1=7, scalar2=None,
                    op0=Alu.arith_shift_right,
                )
                qs1 = misc_pool.tile([P, 32], i32, name=f"qs1{h}")
                nc.vector.tensor_scalar(
                    out=qs1[:], in0=S32, scalar1=7, scalar2=None,
                    op0=Alu.arith_shift_right,
                )
                dq = misc_pool.tile([P, 32], i32, name=f"dq{h}")
                nc.vector.tensor_tensor(
                    out=dq[:], in0=qe1[:], in1=qs1[:], op=Alu.subtract
                )
                qe = misc_pool.tile([P, 32], i32, name=f"qe{h}")
                nc.vector.tensor_scalar(
                    out=qe[:], in0=qe1[:], scalar1=-1, scalar2=None, op0=Alu.add
                )
                gb = misc_pool.tile([P, 32], i32, name=f"gb{h}")
                nc.vector.tensor_tensor(out=gb[:], in0=dq[:], in1=qe[:], op=Alu.mult)
                nc.vector.tensor_scalar(
                    out=gb[:], in0=gb[:], scalar1=128, scalar2=None, op0=Alu.mult
                )
                nc.vector.tensor_scalar(
                    out=gb[:], in0=gb[:], scalar1=127, scalar2=None, op0=Alu.add
                )
                nc.vector.tensor_copy(
                    out=idx[:, 96 * h + 64 : 96 * h + 96], in_=gb[:]
                )

            # widths -> 1/w in output layout (span n=c*128+p at [p, c])
            V = misc_pool.tile([P, 2 * JG], i32)
            nc.sync.dma_start(out=V[:], in_=vse[:])
            wi = misc_pool.tile([P, JG], i32)
            nc.vector.tensor_tensor(
                out=wi[:], in0=V[:, 1 : 2 * JG : 2], in1=V[:, 0 : 2 * JG : 2],
                op=Alu.subtract,
            )
            wf = misc_pool.tile([P, JG], f32)
            nc.vector.tensor_copy(out=wf[:], in_=wi[:])
            wrec = misc_pool.tile([P, JG], f32)
            nc.vector.reciprocal(out=wrec[:], in_=wf[:])

            # --- phase 1: fp16 rank-stripe prefix table in SBUF ---
            cum = misc_pool.tile([P, NRANK * D], f16)
            nc.vector.memset(cum[:, 0:D], 0.0)

            MBK = 8
            for g in range(NBLK // MBK):
                t0 = g * MBK * P
                bigx = x_pool.tile([P, MBK * D], f32r)
                nc.sync.dma_start(
                    out=bigx[:],
                    in_=seq[t0 : t0 + MBK * P, :].rearrange(
                        "(m p) d -> p m d", p=P
                    ).bitcast(f32r),
                )
                for m in range(MBK):
                    k = g * MBK + m
                    ps = ps_pool.tile([P, D], f32)
                    nc.tensor.matmul(
                        out=ps[:],
                        lhsT=tri[:],
                        rhs=bigx[:, m * D : (m + 1) * D],
                        start=True, stop=True,
                    )
                    dst = cum[:, (k + 1) * D : (k + 2) * D]
                    if m % 2 == 0:
                        nc.vector.tensor_copy(out=dst, in_=ps[:])
                    else:
                        nc.scalar.activation(out=dst, in_=ps[:], func=Act.Copy)

            # --- phase 2: chunked SWDGE gathers from SBUF, fp16, transposed ---
            # 6 chunks of 512 idxs (the SWDGE ring rejects >=1024 idxs per
            # instruction); chunk order CE_h, CS_h, CR_h per half so each
            # half combines as soon as its 3 chunks land. Queues rotate.
            CH = 512
            G = misc_pool.tile([P, 6, 2, CH], f16)
            for q in range(6):
                nc.gpsimd.dma_gather(
                    G[:, q],
                    cum[:],
                    idx[:, q * (CH // 16) : (q + 1) * (CH // 16)],
                    CH,
                    CH,
                    elem_size=D,
                    transpose=True,
                    sbuf_tokens_per_rank=P,
                    sbuf_free_dim_per_rank=D * 2,
                    queue_num=q % 4,
                )

            sumT = misc_pool.tile([P, 2, N_SPANS], f16)
            for h in range(2):
                sv = sumT[:, :, h * CH : (h + 1) * CH]
                nc.vector.tensor_tensor(
                    out=sv, in0=G[:, 3 * h], in1=G[:, 3 * h + 1], op=Alu.subtract
                )
                nc.vector.tensor_tensor(
                    out=sv, in0=sv, in1=G[:, 3 * h + 2], op=Alu.add
                )

            # --- transpose back per 128-span chunk, scale by 1/w, store ---
            for j in range(JG):
                rj = res_pool.tile([P, D], f32)
                for c in range(2):
                    pst = pst_pool.tile([P, P], f16)
                    nc.tensor.transpose(
                        out=pst[:],
                        in_=sumT[:, c, j * P : (j + 1) * P],
                        identity=identh[:],
                    )
                    nc.vector.tensor_scalar_mul(
                        out=rj[:, c * P : (c + 1) * P],
                        in0=pst[:],
                        scalar1=wrec[:, j : j + 1],
                    )
                oj = out[:].rearrange("(c p) d -> p c d", p=P)[:, j, :]
                nc.scalar.dma_start(out=oj, in_=rj[:])
    nc.finalize()
    return nc


def _make_in_maps(sequence_tensor, span_indices):
    seq = np.ascontiguousarray(np.asarray(sequence_tensor), dtype=np.float32)
    si = np.asarray(span_indices)
    assert seq.shape == (B, S, D) and si.shape == (B, N_SPANS, 2)
    in_maps = []
    for b in range(B):
        s_vals = si[b, :, 0].astype(np.int32)
        e_vals = si[b, :, 1].astype(np.int32)
        sew = np.zeros((P, 128), dtype=np.int32)
        # per half h of 512 spans: 32 slots e, 32 slots s; within-half
        # logical t -> [t%16 (replicated mod 16), t//16]
        for h in range(2):
            eh = e_vals[h * 512 : (h + 1) * 512]
            sh = s_vals[h * 512 : (h + 1) * 512]
            sew[:, 64 * h : 64 * h + 32] = np.tile(eh.reshape(32, 16).T, (8, 1))
            sew[:, 64 * h + 32 : 64 * h + 64] = np.tile(sh.reshape(32, 16).T, (8, 1))
        # span n = c*128+p at [p, (2c, 2c+1)]
        vse = np.ascontiguousarray(
            si[b].astype(np.int32).reshape(JG, P, 2).transpose(1, 0, 2).reshape(P, 2 * JG)
        )
        in_maps.append({"seq": seq[b], "sew": sew, "vse": vse, "trid": _TRI})
    return in_maps


def kernel(sequence_tensor, span_indices):
    from concourse.bass_utils import run_bass_kernel_spmd

    global _cached_nc
    if _cached_nc is None:
        _cached_nc = build_nc()
    in_maps = _make_in_maps(sequence_tensor, span_indices)
    res = run_bass_kernel_spmd(_cached_nc, in_maps, list(range(B)))
    return np.stack([res.results[b]["out"] for b in range(B)], axis=0)


# revision 21
# speedup vs baseline: 1.3472x; 1.3472x over previous
"""Trainium2 Bass kernel for AverageSpanExtractor (segment mean over spans).

Math note: the reference's masked softmax over all-ones logits reduces
exactly to a mean over the span tokens [start, end):
    out[b, n, :] = mean(sequence_tensor[b, start:end, :]).

Strategy (8 cores, batch-parallel — one batch element per core), built
around sorted-span segment matmuls instead of prefix sums + gathers
(measured: ANY indexed fetch of 3k rows costs >=20us on this part —
SWDGE desc-gen ~9ns/idx, ap_gather/indirect_copy ~30ns/idx):

  1. HOST: sort each batch's spans by start. A 128-span chunk of the
     sorted order covers a ~640-token window, i.e. 5-7 of the 32
     128-token blocks. Window bounds (B0_j, K_j) are unioned across
     the 8 cores so one SPMD program serves all; the nc is built (and
     cached) per span-structure, so bounds are exact for the given
     inputs, correct for any.
  2. DEVICE: per (chunk j, window block b), build the binary indicator
     M[i, t] = (s_i <= t < e_i) with two fused DVE/gpsimd compare ops
     against an iota row (host supplies per-window shifted bounds),
     PE-transpose it to token-major, and accumulate
        out_j += M_T.T @ x_block          (f16 inputs, f32 PSUM)
     Sequence blocks are DMA-streamed f32 and cast f16 on the scalar/
     vector engines; chunk matmuls chase the loads.
  3. Scale rows by 1/w (f32, per-partition) during the PSUM->SBUF
     copy, store contiguous (sorted order). HOST: unpermute rows.

Precision: binary f16 indicator is exact; x quantized to f16
(2^-11) => ~3e-4 global rel err. No prefix-difference cancellation.
"""

import numpy as np

B, S, D = 8, 4096, 256
N_SPANS = 1024
P = 128
NBLK = S // P
JG = N_SPANS // P      # 8 span chunks of 128

_cache = {"key": None, "nc": None, "windows": None}


def _plan_windows(si):
    """Per-chunk sorted-span block windows, unioned across cores.

    Returns (perms [B,1024], windows: list per j of (B0, K)), plus
    sorted s/e arrays [B, 1024].
    """
    perms = np.empty((B, N_SPANS), dtype=np.int64)
    ss = np.empty((B, N_SPANS), dtype=np.int64)
    ee = np.empty((B, N_SPANS), dtype=np.int64)
    for b in range(B):
        perm = np.argsort(si[b, :, 0], kind="stable")
        perms[b] = perm
        ss[b] = si[b, perm, 0]
        ee[b] = si[b, perm, 1]
    windows = []
    for j in range(JG):
        b0 = NBLK
        b1 = 0
        for b in range(B):
            cs = ss[b, j * P : (j + 1) * P]
            ce = ee[b, j * P : (j + 1) * P]
            b0 = min(b0, int(cs.min()) >> 7)
            b1 = max(b1, (int(ce.max()) - 1) >> 7)
        windows.append((b0, b1 - b0 + 1))
    return perms, windows, ss, ee


def build_nc(windows):
    import concourse.bacc as bacc
    import concourse.mybir as mybir
    from concourse.tile import TileContext
    from concourse.masks import make_identity

    f32 = mybir.dt.float32
    f16 = mybir.dt.float16
    Alu = mybir.AluOpType
    Act = mybir.ActivationFunctionType

    NW = sum(k for _, k in windows)

    nc = bacc.Bacc(None, target_bir_lowering=False, debug=False, num_devices=B)
    seq = nc.declare_dram_parameter("seq", [S, D], f32, isOutput=False)
    # Per-window shifted span bounds (f32): column w of window (j, b)
    # holds s_sorted[128j+p] - 128*(B0_j+b) (resp. e).
    swin = nc.declare_dram_parameter("swin", [P, NW], f32, isOutput=False)
    ewin = nc.declare_dram_parameter("ewin", [P, NW], f32, isOutput=False)
    # 1/w per sorted span, chunk-major: [p, j].
    wrec = nc.declare_dram_parameter("wrec", [P, JG], f32, isOutput=False)
    out = nc.declare_dram_parameter("out", [N_SPANS, D], f32, isOutput=True)

    with TileContext(nc) as tc:
        with (
            tc.tile_pool(name="const", bufs=1) as const_pool,
            tc.tile_pool(name="x", bufs=3) as x_pool,
            tc.tile_pool(name="m", bufs=6) as m_pool,
            tc.tile_pool(name="ps", bufs=4, space="PSUM") as ps_pool,
            tc.tile_pool(name="pst", bufs=4, space="PSUM") as pst_pool,
            tc.tile_pool(name="misc", bufs=1) as misc_pool,
            tc.tile_pool(name="res", bufs=3) as res_pool,
        ):
            identh = const_pool.tile([P, P], f16)
            make_identity(nc, identh[:])
            iota = const_pool.tile([P, P], mybir.dt.int32)
            nc.gpsimd.iota(iota[:], pattern=[[1, P]], base=0, channel_multiplier=0)
            iotaF = const_pool.tile([P, P], f32)
            nc.gpsimd.tensor_copy(out=iotaF[:], in_=iota[:])

            SW = misc_pool.tile([P, NW], f32)
            nc.sync.dma_start(out=SW[:], in_=swin[:])
            EW = misc_pool.tile([P, NW], f32)
            nc.sync.dma_start(out=EW[:], in_=ewin[:])
            WR = misc_pool.tile([P, JG], f32)
            nc.sync.dma_start(out=WR[:], in_=wrec[:])

            # indicator build + transpose, all before the compute matmuls
            MTbig = misc_pool.tile([P, NW, P], f16)
            w = 0
            for j in range(JG):
                b0, kj = windows[j]
                for bb in range(kj):
                    A = m_pool.tile([P, P], f16, name=f"A{w}")
                    nc.vector.tensor_scalar(
                        out=A[:], in0=iotaF[:], scalar1=SW[:, w : w + 1],
                        scalar2=None, op0=Alu.is_ge,
                    )
                    M = m_pool.tile([P, P], f16, name=f"M{w}")
                    nc.vector.scalar_tensor_tensor(
                        out=M[:], in0=iotaF[:], scalar=EW[:, w : w + 1],
                        in1=A[:], op0=Alu.is_lt, op1=Alu.mult,
                    )
                    pst = pst_pool.tile([P, P], f16)
                    nc.tensor.transpose(out=pst[:], in_=M[:], identity=identh[:])
                    nc.scalar.activation(
                        out=MTbig[:, w, :], in_=pst[:], func=Act.Copy
                    )
                    w += 1

            # stream x: 8 groups of 4 blocks (512 KB); cast f16
            GB = 4
            XH = misc_pool.tile([P, NBLK * D], f16)
            for g in range(NBLK // GB):
                t0 = g * GB * P
                bigx = x_pool.tile([P, GB * D], f32)
                nc.sync.dma_start(
                    out=bigx[:],
                    in_=seq[t0 : t0 + GB * P, :].rearrange("(m p) d -> p m d", p=P),
                )
                xsl = XH[:, t0 * 2 : (t0 + GB * P) * 2]
                if g % 2 == 0:
                    nc.scalar.activation(out=xsl, in_=bigx[:], func=Act.Copy)
                else:
                    nc.vector.tensor_copy(out=xsl, in_=bigx[:])

            # chunk accumulation matmuls, then scale + store
            w = 0
            for j in range(JG):
                b0, kj = windows[j]
                ps = ps_pool.tile([P, D], f32)
                for bb in range(kj):
                    blk = b0 + bb
                    nc.tensor.matmul(
                        out=ps[:],
                        lhsT=MTbig[:, w, :],
                        rhs=XH[:, blk * D : (blk + 1) * D],
                        start=(bb == 0), stop=(bb == kj - 1),
                    )
                    w += 1
                rj = res_pool.tile([P, D], f32)
                nc.vector.tensor_scalar_mul(
                    out=rj[:], in0=ps[:], scalar1=WR[:, j : j + 1]
                )
                oj = out[:].rearrange("(c p) d -> p c d", p=P)[:, j, :]
                nc.scalar.dma_start(out=oj, in_=rj[:])
    nc.finalize()
    return nc


def _make_in_maps(sequence_tensor, si, perms, windows, ss, ee):
    seq = np.ascontiguousarray(np.asarray(sequence_tensor), dtype=np.float32)
    NW = sum(k for _, k in windows)
    in_maps = []
    for b in range(B):
        sw = np.empty((P, NW), dtype=np.float32)
        ew = np.empty((P, NW), dtype=np.float32)
        w = 0
        for j in range(JG):
            b0, kj = windows[j]
            cs = ss[b, j * P : (j + 1) * P].astype(np.float32)
            ce = ee[b, j * P : (j + 1) * P].astype(np.float32)
            for bb in range(kj):
                base = 128.0 * (b0 + bb)
                sw[:, w] = cs - base
                ew[:, w] = ce - base
                w += 1
        wr = (
            1.0
            / (ee[b] - ss[b]).astype(np.float32)
        ).reshape(JG, P).T.copy()
        in_maps.append({"seq": seq[b], "swin": sw, "ewin": ew, "wrec": wr})
    return in_maps


def kernel(sequence_tensor, span_indices):
    from concourse.bass_utils import run_bass_kernel_spmd

    si = np.asarray(span_indices)
    assert si.shape == (B, N_SPANS, 2)
    key = si.tobytes()
    if _cache["key"] != key:
        perms, windows, ss, ee = _plan_windows(si)
        _cache.update(
            key=key, nc=build_nc(windows),
            plan=(perms, windows, ss, ee),
        )
    perms, windows, ss, ee = _cache["plan"]
    in_maps = _make_in_maps(sequence_tensor, si, perms, windows, ss, ee)
    res = run_bass_kernel_spmd(_cache["nc"], in_maps, list(range(B)))
    full = np.empty((B, N_SPANS, D), dtype=np.float32)
    for b in range(B):
        # device row i (sorted order) -> original span perms[b][i]
        full[b, perms[b], :] = res.results[b]["out"]
    return full
